# revision 18
# baseline (speedup 1.0000x reference)
"""Trainium2 Bass kernel for nn_CATKT (embedding + LSTM + cumulative-softmax
attention + fc), data-parallel over batch across 8 NeuronCores.

Math notes (exact reformulations of the reference):
  - sae construction: top half = answer==1 ? skill_emb : answer_table[0],
    bottom half = answer==1 ? answer_table[1] : skill_emb. Both halves are a
    single row-gather from an augmented table [skill_table; T0; T1] with
    host-fused indices.
  - The (B,L,L) attention collapses: scores row i is att[j] masked to j<=i, so
    softmax weights are exp(att_j)/cumsum(exp(att_j)) and
    attn_out[t] = r_t with r_t = (1-p_t) r_{t-1} + p_t h_t,
    p_t = w_t / cumsum(w)_t, w_t = exp(att_t)  — a linear scan.
  - attn_cum_1 = exclusive cumsum of attn_out; fc is applied as
    logits.T = Wc @ attn_cum.T (fp32) + Wh @ h.T (bf16) + fc_b.

Per-core layout: 4 sequences, tokens interleaved tok = 4*t + b, everything
stored feature-major [feature-part, token-free]. Pipeline over 8 chunks of
512 tokens (128 LSTM steps each).
"""
import sys

sys.path.insert(0, "/opt/trn_rl_repo")

import numpy as np
import ml_dtypes

import concourse.bass as bass
import concourse.tile as tile
from concourse import mybir
from concourse.bass_utils import run_bass_kernel_spmd

F32 = mybir.dt.float32
BF16 = mybir.dt.float16  # fp16: same PE speed as bf16, 8x mantissa
I32 = mybir.dt.int32
AF = mybir.ActivationFunctionType
OP = mybir.AluOpType

B, L = 32, 1024
NCORES = 8
BSH = B // NCORES          # 4 sequences per core
SKILL_DIM = 256
IN_DIM = 512
H = 512
GATES = 2048
ATTN = 80
NCLS = 1000
CH = 512                   # tokens per chunk
CSTEPS = CH // BSH         # 128 LSTM steps per chunk
NCHUNK = int(__import__("os").environ.get("NCHUNK", (L * BSH) // CH))
CT = 125                   # class tile (8 x 125 = 1000)


def _split_waits(nc):
    """This walrus build rejects >1 sync-wait per instruction; hoist extras
    onto NoOps inserted just before the carrying instruction."""
    for fn in nc.m.functions:
        for bb in fn.blocks:
            insts = bb.instructions
            i = 0
            while i < len(insts):
                inst = insts[i]
                si = getattr(inst, "sync_info", None)
                if si is not None and si.on_wait is not None and len(si.on_wait) > 1:
                    waits = list(si.on_wait)
                    nops = []
                    for w in waits[:-1]:
                        nop = mybir.InstNoOp(
                            name=nc.get_next_instruction_name(),
                            engine=inst.engine, ins=[], outs=[],
                        )
                        nop.sync_info = mybir.SyncInfo(on_wait=[w], on_update=[])
                        nc.register_instruction(nop)
                        nops.append(nop)
                    inst.sync_info = mybir.SyncInfo(
                        on_wait=[waits[-1]], on_update=list(si.on_update or [])
                    )
                    insts[i:i] = nops
                    i += len(nops)
                i += 1


def build_nc():
    nc = bass.Bass()
    d = {}
    d["table2"] = nc.dram_tensor("table2", [1003, SKILL_DIM], F32, kind="ExternalInput")
    d["idx_top"] = nc.dram_tensor("idx_top", [NCHUNK * 4, 128], I32, kind="ExternalInput")
    d["idx_bot"] = nc.dram_tensor("idx_bot", [NCHUNK * 4, 128], I32, kind="ExternalInput")
    d["wih"] = nc.dram_tensor("wih", [IN_DIM, GATES], BF16, kind="ExternalInput")   # W_ih.T
    d["whh"] = nc.dram_tensor("whh", [H, GATES], BF16, kind="ExternalInput")        # W_hh.T
    d["bias"] = nc.dram_tensor("bias", [16, 128], F32, kind="ExternalInput")        # b_ih+b_hh
    d["mlpw"] = nc.dram_tensor("mlpw", [H, ATTN], BF16, kind="ExternalInput")       # mlp_W.T
    d["mlpb"] = nc.dram_tensor("mlpb", [ATTN, 1], F32, kind="ExternalInput")
    d["simw"] = nc.dram_tensor("simw", [ATTN, 1], F32, kind="ExternalInput")        # sim_W.T
    d["wc"] = nc.dram_tensor("wc", [H, NCLS], F32, kind="ExternalInput")            # fc_W[:, :H].T
    d["wh"] = nc.dram_tensor("wh", [H, NCLS], BF16, kind="ExternalInput")           # fc_W[:, H:].T
    d["fcb"] = nc.dram_tensor("fcb", [8, CT], F32, kind="ExternalInput")
    res = nc.dram_tensor("res", [NCLS, BSH * L], F32, kind="ExternalOutput")
    import os as _os
    dbg = None
    if _os.environ.get("KDEBUG") == "1":
        dbg = {
            "hst": nc.dram_tensor("dbg_hst", [512, BSH * L], F32, kind="ExternalOutput"),
            "attn": nc.dram_tensor("dbg_attn", [512, BSH * L], F32, kind="ExternalOutput"),
            "acum": nc.dram_tensor("dbg_acum", [512, BSH * L], F32, kind="ExternalOutput"),
            "xpre": nc.dram_tensor("dbg_xpre", [GATES, CH], F32, kind="ExternalOutput"),
            "wv": nc.dram_tensor("dbg_wv", [NCHUNK, CH], F32, kind="ExternalOutput"),
        }
    pscr = nc.dram_tensor("pscr", [NCHUNK, CH], F32)  # bounce buffer for p broadcast

    with tile.TileContext(nc) as tc:
        build_tile(nc, tc, d, res, pscr, dbg)
    _split_waits(nc)
    return nc


def build_tile(nc, tc, d, res, pscr, dbg=None):
    from contextlib import ExitStack

    ctx = ExitStack()
    weights = ctx.enter_context(tc.tile_pool(name="weights", bufs=1))
    state = ctx.enter_context(tc.tile_pool(name="state", bufs=1))
    chunkp = ctx.enter_context(tc.tile_pool(name="chunkp", bufs=2))
    small = ctx.enter_context(tc.tile_pool(name="small", bufs=2))
    xpp = ctx.enter_context(tc.tile_pool(name="xpp", bufs=1))
    atmp = ctx.enter_context(tc.tile_pool(name="atmp", bufs=1))
    psum = ctx.enter_context(tc.tile_pool(name="psum", bufs=3, space="PSUM"))
    psg = ctx.enter_context(tc.tile_pool(name="psg", bufs=1, space="PSUM"))

    # ---- load weights ----
    wih = weights.tile([128, 4 * GATES], BF16)    # k-tile k at cols [k*2048, +2048]
    nc.sync.dma_start(out=wih[:].rearrange("p (k g) -> p k g", k=4), in_=d["wih"][:].rearrange("(k p) g -> p k g", p=128))
    whh = weights.tile([128, 4 * GATES], BF16)
    nc.sync.dma_start(out=whh[:].rearrange("p (k g) -> p k g", k=4), in_=d["whh"][:].rearrange("(k p) g -> p k g", p=128))
    wc = weights.tile([128, 4 * NCLS], F32)
    nc.sync.dma_start(out=wc[:].rearrange("p (k g) -> p k g", k=4), in_=d["wc"][:].rearrange("(k p) g -> p k g", p=128))
    wh = weights.tile([128, 4 * NCLS], BF16)
    nc.sync.dma_start(out=wh[:].rearrange("p (k g) -> p k g", k=4), in_=d["wh"][:].rearrange("(k p) g -> p k g", p=128))
    mlpw = weights.tile([128, 4 * ATTN], BF16)
    nc.sync.dma_start(out=mlpw[:].rearrange("p (k g) -> p k g", k=4), in_=d["mlpw"][:].rearrange("(k p) g -> p k g", p=128))
    bias = weights.tile([128, 16], F32)
    nc.sync.dma_start(out=bias[:], in_=d["bias"][:].rearrange("m p -> p m"))
    mlpb = weights.tile([ATTN, 1], F32)
    nc.sync.dma_start(out=mlpb[:], in_=d["mlpb"][:])
    simw = weights.tile([ATTN, 1], F32)
    nc.sync.dma_start(out=simw[:], in_=d["simw"][:])
    fcb = weights.tile([CT, 8], F32)
    nc.sync.dma_start(out=fcb[:], in_=d["fcb"][:].rearrange("m p -> p m"))
    ones80 = weights.tile([ATTN, 1], F32)
    nc.vector.memset(ones80[:], 1.0)

    # ---- persistent LSTM state ----
    h_bf = state.tile([128, 16], BF16)     # 4 hid-chunks x 4 batch
    c_st = state.tile([128, 16], F32)
    nc.vector.memset(h_bf[:], 0.0)
    nc.vector.memset(c_st[:], 0.0)
    rcarry = state.tile([128, 16], F32)    # attn_out carry per (hidchunk, seq)
    acarry = state.tile([128, 16], F32)    # inclusive-cumsum carry
    dcarry = state.tile([1, 4], F32)       # softmax denominator carry
    nc.vector.memset(rcarry[:], 0.0)
    nc.vector.memset(acarry[:], 0.0)
    nc.vector.memset(dcarry[:], 0.0)

    gates_ps = psg.tile([128, 64], F32)    # LSTM gate psum: m-tile m at [4m:4m+4]

    for c in range(NCHUNK):
        # ================= phase A: gather + transpose + x_pre =================
        saeT = chunkp.tile([128, 4 * CH], BF16, tag="saeT")  # in-chunk k at [k*CH,+CH]
        for g in range(4):
            for which, idxd in (("t", d["idx_top"]), ("b", d["idx_bot"])):
                it = small.tile([128, 1], I32, tag="it")
                nc.sync.dma_start(out=it[:], in_=idxd[4 * c + g:4 * c + g + 1, :])
                gt = small.tile([128, SKILL_DIM], F32, tag="gt")
                nc.gpsimd.indirect_dma_start(
                    out=gt[:], out_offset=None, in_=d["table2"][:],
                    in_offset=bass.IndirectOffsetOnAxis(ap=it[:, :1], axis=0),
                )
                gb = small.tile([128, SKILL_DIM], BF16, tag="gb")
                nc.vector.tensor_copy(out=gb[:], in_=gt[:])
                koff = 0 if which == "t" else 2
                for j in range(2):
                    nc.sync.dma_start_transpose(
                        out=saeT[:, (koff + j) * CH + g * 128:(koff + j) * CH + (g + 1) * 128],
                        in_=gb[:, j * 128:(j + 1) * 128],
                    )

        xpreT = xpp.tile([128, 16 * CH], F32, tag="xpreT")  # m-tile m at [m*CH,+CH]
        for m in range(16):
            xps = psum.tile([128, CH], F32, tag="pp")
            for k in range(4):
                nc.tensor.matmul(
                    out=xps[:],
                    lhsT=wih[:, k * GATES + m * 128:k * GATES + (m + 1) * 128],
                    rhs=saeT[:, k * CH:(k + 1) * CH],
                    start=(k == 0), stop=(k == 3),
                )
            nc.scalar.activation(out=xpreT[:, m * CH:(m + 1) * CH], in_=xps[:],
                                 func=AF.Identity, bias=bias[:, m:m + 1])

        # ================= phase B: LSTM (CSTEPS steps) =================
        hstore = xpp.tile([128, 4 * CH], F32, tag="hstore")
        hstore_bf = chunkp.tile([128, 4 * CH], BF16, tag="hstore_bf")
        xpre3 = xpreT.rearrange("p (m w) -> p m w", m=16)
        hst3 = hstore.rearrange("p (k w) -> p k w", k=4)
        hstb3 = hstore_bf.rearrange("p (k w) -> p k w", k=4)
        gp3 = gates_ps[:].rearrange("p (m n) -> p m n", m=16)

        with tc.For_i(0, CH, 4, staggered_reset=(__import__("os").environ.get("STAGGER","1")=="1")) as t:
            for m in range(16):
                for k in range(4):
                    nc.tensor.matmul(
                        out=gates_ps[:, 4 * m:4 * m + 4],
                        lhsT=whh[:, k * GATES + m * 128:k * GATES + (m + 1) * 128],
                        rhs=h_bf[:, 4 * k:4 * k + 4],
                        start=(k == 0), stop=(k == 3),
                    )
            gsum = small.tile([128, 64], F32, tag="gsum")
            nc.vector.tensor_tensor(
                out=gsum[:].rearrange("p (m n) -> p m n", m=16),
                in0=gp3, in1=xpre3[:, :, bass.ds(t, 4)], op=OP.add,
            )
            gsig = small.tile([128, 64], F32, tag="gsig")
            nc.scalar.activation(out=gsig[:, 0:32], in_=gsum[:, 0:32], func=AF.Sigmoid)
            nc.scalar.activation(out=gsig[:, 32:48], in_=gsum[:, 32:48], func=AF.Tanh)
            nc.scalar.activation(out=gsig[:, 48:64], in_=gsum[:, 48:64], func=AF.Sigmoid)
            t1 = small.tile([128, 16], F32, tag="t1")
            nc.vector.tensor_tensor(out=t1[:], in0=gsig[:, 0:16], in1=gsig[:, 32:48], op=OP.mult)
            nc.vector.tensor_tensor(out=c_st[:], in0=gsig[:, 16:32], in1=c_st[:], op=OP.mult)
            nc.vector.tensor_tensor(out=c_st[:], in0=c_st[:], in1=t1[:], op=OP.add)
            tc_t = small.tile([128, 16], F32, tag="tc_t")
            nc.scalar.activation(out=tc_t[:], in_=c_st[:], func=AF.Tanh)
            hprod = small.tile([128, 16], F32, tag="hprod")
            nc.vector.tensor_tensor(out=hprod[:], in0=gsig[:, 48:64], in1=tc_t[:], op=OP.mult)
            nc.vector.tensor_copy(out=h_bf[:], in_=hprod[:])
            hp3 = hprod[:].rearrange("p (k n) -> p k n", k=4)
            nc.vector.tensor_copy(out=hst3[:, :, bass.ds(t, 4)], in_=hp3)

        nc.vector.tensor_copy(out=hstore_bf[:], in_=hstore[:])

        # ================= phase C: attention =================
        # att scalar per token
        aps = psum.tile([ATTN, CH], F32, tag="pp")
        for k in range(4):
            nc.tensor.matmul(
                out=aps[:],
                lhsT=mlpw[:, k * ATTN:(k + 1) * ATTN],
                rhs=hstore_bf[:, k * CH:(k + 1) * CH],
                start=(k == 0), stop=(k == 3),
            )
        tm = atmp.tile([ATTN, CH], F32, tag="tm")
        nc.scalar.activation(out=tm[:], in_=aps[:], func=AF.Tanh, bias=mlpb[:, :1])
        nc.vector.tensor_scalar_mul(out=tm[:], in0=tm[:], scalar1=simw[:, :1])
        att_ps = psum.tile([1, CH], F32, tag="pp")
        nc.tensor.matmul(out=att_ps[:], lhsT=ones80[:], rhs=tm[:], start=True, stop=True)
        wv = atmp.tile([1, CH], F32, tag="wv")
        nc.scalar.activation(out=wv[:], in_=att_ps[:], func=AF.Exp)

        # denominator scan + p
        den = atmp.tile([1, CH], F32, tag="den")
        for b_ in range(4):
            nc.vector.tensor_tensor_scan(
                out=den[:, b_:CH:4], data0=wv[:, b_:CH:4], data1=wv[:, b_:CH:4],
                initial=dcarry[:, b_:b_ + 1], op0=OP.add, op1=OP.bypass,
            )
        nc.vector.tensor_copy(out=dcarry[:], in_=den[:, CH - 4:CH])
        rden = atmp.tile([1, CH], F32, tag="rden")
        nc.vector.reciprocal(out=rden[:], in_=den[:])
        pvec = atmp.tile([1, CH], F32, tag="pvec")
        nc.vector.tensor_tensor(out=pvec[:], in0=wv[:], in1=rden[:], op=OP.mult)
        # broadcast p to 128 partitions via DRAM bounce
        nc.sync.dma_start(out=pscr[c:c + 1, :], in_=pvec[:])
        pb = atmp.tile([128, CH], F32, tag="pb")
        nc.gpsimd.dma_start(
            out=pb[:],
            in_=bass.AP(pscr, c * CH, [[0, 128], [1, CH]]),
        )
        omp = atmp.tile([128, CH], F32, tag="omp")
        nc.vector.tensor_scalar(out=omp[:], in0=pb[:], scalar1=-1.0, scalar2=1.0,
                                op0=OP.mult, op1=OP.add)
        ph = atmp.tile([128, 4 * CH], F32, tag="ph")
        for k in range(4):
            nc.vector.tensor_tensor(out=ph[:, k * CH:(k + 1) * CH], in0=pb[:],
                                    in1=hstore[:, k * CH:(k + 1) * CH], op=OP.mult)
        # r-scan: attn_out
        attn = chunkp.tile([128, 4 * CH], F32, tag="attn")
        for k in range(4):
            for b_ in range(4):
                nc.vector.tensor_tensor_scan(
                    out=attn[:, k * CH + b_:(k + 1) * CH:4],
                    data0=omp[:, b_:CH:4],
                    data1=ph[:, k * CH + b_:(k + 1) * CH:4],
                    initial=rcarry[:, 4 * k + b_:4 * k + b_ + 1],
                    op0=OP.mult, op1=OP.add,
                )
        at3 = attn.rearrange("p (k w) -> p k w", k=4)
        nc.vector.tensor_copy(out=rcarry[:].rearrange("p (k n) -> p k n", k=4),
                              in_=at3[:, :, CH - 4:CH])
        # inclusive cumsum with 4-col carry prefix -> exclusive view
        acum = chunkp.tile([128, 4 * (CH + 4)], F32, tag="acum")
        ac3 = acum.rearrange("p (k w) -> p k w", k=4)
        nc.vector.tensor_copy(out=ac3[:, :, 0:4], in_=acarry[:].rearrange("p (k n) -> p k n", k=4))
        for k in range(4):
            for b_ in range(4):
                nc.vector.tensor_tensor_scan(
                    out=acum[:, k * (CH + 4) + 4 + b_:k * (CH + 4) + 4 + CH:4],
                    data0=attn[:, k * CH + b_:(k + 1) * CH:4],
                    data1=attn[:, k * CH + b_:(k + 1) * CH:4],
                    initial=acum[:, k * (CH + 4) + b_:k * (CH + 4) + b_ + 1],
                    op0=OP.add, op1=OP.bypass,
                )
        nc.vector.tensor_copy(out=acarry[:].rearrange("p (k n) -> p k n", k=4),
                              in_=ac3[:, :, CH:CH + 4])

        if dbg is not None:
            for k in range(4):
                nc.sync.dma_start(out=dbg["hst"][k * 128:(k + 1) * 128, c * CH:(c + 1) * CH], in_=hstore[:, k * CH:(k + 1) * CH])
                nc.sync.dma_start(out=dbg["attn"][k * 128:(k + 1) * 128, c * CH:(c + 1) * CH], in_=attn[:, k * CH:(k + 1) * CH])
                nc.sync.dma_start(out=dbg["acum"][k * 128:(k + 1) * 128, c * CH:(c + 1) * CH], in_=acum[:, k * (CH + 4):k * (CH + 4) + CH])
            nc.sync.dma_start(out=dbg["wv"][c:c + 1, :], in_=wv[:])
            if c == 0:
                for m in range(16):
                    nc.sync.dma_start(out=dbg["xpre"][m * 128:(m + 1) * 128, :], in_=xpreT[:, m * CH:(m + 1) * CH])

        # ================= phase D: fc + sigmoid + store =================
        for ctI in range(8):
            lps = psum.tile([CT, CH], F32, tag="pp")
            for k in range(4):
                nc.tensor.matmul(
                    out=lps[:],
                    lhsT=wc[:, k * NCLS + ctI * CT:k * NCLS + (ctI + 1) * CT],
                    rhs=acum[:, k * (CH + 4):k * (CH + 4) + CH],
                    start=(k == 0), stop=False,
                )
            for k in range(4):
                nc.tensor.matmul(
                    out=lps[:],
                    lhsT=wh[:, k * NCLS + ctI * CT:k * NCLS + (ctI + 1) * CT],
                    rhs=hstore_bf[:, k * CH:(k + 1) * CH],
                    start=False, stop=(k == 3),
                )
            rs = small.tile([CT, CH], F32, tag="rs")
            nc.scalar.activation(out=rs[:], in_=lps[:], func=AF.Sigmoid,
                                 bias=fcb[:, ctI:ctI + 1])
            nc.sync.dma_start(out=res[ctI * CT:(ctI + 1) * CT, c * CH:(c + 1) * CH],
                              in_=rs[:])
    ctx.close()


_NC_CACHE = None
_LAST_RESULTS = None


def build_null_nc():
    """Same ExternalInput/Output declarations as build_nc but a trivial body —
    used to measure the fixed PJRT/axon dispatch + transfer overhead."""
    nc = bass.Bass()
    nc.dram_tensor("table2", [1003, SKILL_DIM], F32, kind="ExternalInput")
    nc.dram_tensor("idx_top", [NCHUNK * 4, 128], I32, kind="ExternalInput")
    nc.dram_tensor("idx_bot", [NCHUNK * 4, 128], I32, kind="ExternalInput")
    nc.dram_tensor("wih", [IN_DIM, GATES], BF16, kind="ExternalInput")
    nc.dram_tensor("whh", [H, GATES], BF16, kind="ExternalInput")
    bias_d = nc.dram_tensor("bias", [16, 128], F32, kind="ExternalInput")
    nc.dram_tensor("mlpw", [H, ATTN], BF16, kind="ExternalInput")
    nc.dram_tensor("mlpb", [ATTN, 1], F32, kind="ExternalInput")
    nc.dram_tensor("simw", [ATTN, 1], F32, kind="ExternalInput")
    nc.dram_tensor("wc", [H, NCLS], F32, kind="ExternalInput")
    nc.dram_tensor("wh", [H, NCLS], BF16, kind="ExternalInput")
    nc.dram_tensor("fcb", [8, CT], F32, kind="ExternalInput")
    res = nc.dram_tensor("res", [NCLS, BSH * L], F32, kind="ExternalOutput")
    with tile.TileContext(nc) as tc:
        with tc.tile_pool(name="p", bufs=1) as pool:
            t = pool.tile([16, 128], F32)
            nc.sync.dma_start(out=t[:], in_=bias_d[:])
            nc.sync.dma_start(out=res[0:16, 0:128], in_=t[:])
    _split_waits(nc)
    return nc


def prepare_in_maps(skill, answer, skill_table, answer_table, W_ih, W_hh, b_ih, b_hh,
                    mlp_W, mlp_b, sim_W, fc_W, fc_b):
    skill = np.asarray(skill)
    answer = np.asarray(answer)
    skill_table = np.asarray(skill_table, dtype=np.float32)
    answer_table = np.asarray(answer_table, dtype=np.float32)
    W_ih = np.asarray(W_ih, dtype=np.float32)
    W_hh = np.asarray(W_hh, dtype=np.float32)
    b = (np.asarray(b_ih, dtype=np.float32) + np.asarray(b_hh, dtype=np.float32))
    mlp_W = np.asarray(mlp_W, dtype=np.float32)
    mlp_b = np.asarray(mlp_b, dtype=np.float32)
    sim_W = np.asarray(sim_W, dtype=np.float32)
    fc_W = np.asarray(fc_W, dtype=np.float32)
    fc_b = np.asarray(fc_b, dtype=np.float32)

    bf = lambda x: np.ascontiguousarray(x).astype(np.float16)
    table2 = np.concatenate([skill_table, answer_table[0:1], answer_table[1:2]], axis=0)
    shared = {
        "table2": np.ascontiguousarray(table2, dtype=np.float32),
        "wih": bf(W_ih.T),
        "whh": bf(W_hh.T),
        "bias": np.ascontiguousarray(b.reshape(16, 128)),
        "mlpw": bf(mlp_W.T),
        "mlpb": np.ascontiguousarray(mlp_b.reshape(ATTN, 1)),
        "simw": np.ascontiguousarray(sim_W.reshape(1, ATTN).T),
        "wc": np.ascontiguousarray(fc_W[:, :H].T, dtype=np.float32),
        "wh": bf(fc_W[:, H:].T),
        "fcb": np.ascontiguousarray(fc_b.reshape(8, CT)),
    }

    in_maps = []
    for core in range(NCORES):
        sb = slice(core * BSH, (core + 1) * BSH)
        sk = skill[sb].astype(np.int32)        # [BSH, L]
        an = answer[sb].astype(np.int32)
        # token order tok = 4*t + b  ->  [L, BSH] flattened
        top = np.where(an == 1, sk, 1001).T.reshape(-1)   # [L*BSH]
        bot = np.where(an == 1, 1002, sk).T.reshape(-1)
        m = dict(shared)
        m["idx_top"] = np.ascontiguousarray(top.reshape(NCHUNK * 4, 128), dtype=np.int32)
        m["idx_bot"] = np.ascontiguousarray(bot.reshape(NCHUNK * 4, 128), dtype=np.int32)
        in_maps.append(m)
    return in_maps


def assemble_output(results):
    out = np.empty((B, L, NCLS), np.float32)
    for core in range(NCORES):
        rc = results[core]["res"]            # [NCLS, BSH*L], tok = 4t+b
        rc = rc.reshape(NCLS, L, BSH)
        out[core * BSH:(core + 1) * BSH] = rc.transpose(2, 1, 0)
    return out


def kernel(**inputs):
    global _NC_CACHE
    in_maps = prepare_in_maps(**inputs)
    if _NC_CACHE is None:
        _NC_CACHE = build_nc()
    r = run_bass_kernel_spmd(_NC_CACHE, in_maps, list(range(NCORES)), trace=False)
    global _LAST_RESULTS
    _LAST_RESULTS = r.results
    return assemble_output(r.results)


if __name__ == "__main__":
    rng = np.random.default_rng(0)
    ins = {
        "skill": rng.integers(0, 1000, (B, L)),
        "answer": rng.integers(0, 2, (B, L)),
        "skill_table": rng.standard_normal((1001, SKILL_DIM), dtype=np.float32) * 0.05,
        "answer_table": rng.standard_normal((3, SKILL_DIM), dtype=np.float32) * 0.05,
        "W_ih": rng.standard_normal((GATES, IN_DIM), dtype=np.float32) * 0.05,
        "W_hh": rng.standard_normal((GATES, H), dtype=np.float32) * 0.05,
        "b_ih": np.zeros(GATES, np.float32),
        "b_hh": np.zeros(GATES, np.float32),
        "mlp_W": rng.standard_normal((ATTN, H), dtype=np.float32) * 0.05,
        "mlp_b": np.zeros(ATTN, np.float32),
        "sim_W": rng.standard_normal((1, ATTN), dtype=np.float32) * 0.05,
        "fc_W": rng.standard_normal((NCLS, 2 * H), dtype=np.float32) * 0.05,
        "fc_b": np.zeros(NCLS, np.float32),
    }
    o = kernel(**ins)
    print(o.shape, o.dtype, float(o.min()), float(o.max()))
